# revision 1
# baseline (speedup 1.0000x reference)
"""Trainium2 Bass kernel for the Context Encoder problem:

    ce  = c2e_weight[nodes]            # [N, 128] embedding gather
    h   = relu(ce @ w1.T + b1)         # [N, 128]
    out = relu(h @ w2.T + b2)          # [N, 128]

Strategy (8 NeuronCores, vocab-range sharding):
  200000 node ids over a 100000-row vocab saturate every vocab window,
  so transforming the table itself is less work than gathering per-node
  rows (and avoids the per-index DMA descriptor-generation cost that
  dominates any on-device gather).

  - The vocab is split into 8 fixed 12500-row ranges.  Core i streams
    its host-pre-transposed (d-major) table window [128, 12800]
    contiguously at full DMA bandwidth and computes
    T2 = relu(relu(win @ w1.T + b1) @ w2.T + b2) for every window row.
  - d-major input feeds mm1 directly (lhsT = w1.T stationary, window as
    the moving operand); mm2 keeps w2.T stationary.  Both layers' biases
    are per-partition, so relu+bias fuses into one ScalarE activation or
    one VectorE dual-op tensor_scalar; the two relus alternate between
    ACT and DVE for engine balance.  No PE transposes, no PSUM staging
    copies, no bias matmuls.
  - Results stay feature-major; the host transposes each window and maps
    node positions to rows (out = T2[nodes]) as the unshard step.
"""

import os
import sys

for _p in ("/opt/trn_rl_repo",):
    if _p not in sys.path:
        sys.path.insert(0, _p)

import numpy as np

import concourse.bass as bass
import concourse.mybir as mybir
import concourse.tile as tile
from concourse import bacc
from concourse.bass_utils import run_bass_kernel_spmd
from concourse.tile import TileContext

P = 128
D = 128
N_CORES = 8
VOCAB = 100000
RANGE = VOCAB // N_CORES   # 12500 vocab rows owned per core
BLOCKS = 100               # 12800 rows processed per core (128*100)
CHUNK_BLOCKS = 20          # rows DMA'd per input chunk (1.31 MB)
G = 4                      # blocks per compute super-tile (free dim 512)


def build_nc(blocks: int = BLOCKS, chunk_blocks: int = CHUNK_BLOCKS,
             g: int = G, use_f32r: bool = False):
    assert blocks % g == 0 and chunk_blocks % g == 0
    f32 = mybir.dt.float32
    nc = bacc.Bacc("TRN2", target_bir_lowering=False, debug=False,
                   num_devices=N_CORES)

    rows = blocks * P
    tsl_t = nc.dram_tensor("tslice", [P, rows], f32,
                           kind="ExternalInput").ap()
    w1t_t = nc.dram_tensor("w1t", [D, D], f32, kind="ExternalInput").ap()
    w2t_t = nc.dram_tensor("w2t", [D, D], f32, kind="ExternalInput").ap()
    b1_t = nc.dram_tensor("b1c", [P, 1], f32, kind="ExternalInput").ap()
    b2_t = nc.dram_tensor("b2c", [P, 1], f32, kind="ExternalInput").ap()
    out_t = nc.dram_tensor("out", [P, rows], f32,
                           kind="ExternalOutput").ap()

    fw = g * D  # super-tile free width (512)

    with TileContext(nc) as tc:
        with (
            tc.tile_pool(name="const", bufs=1) as cpool,
            tc.tile_pool(name="win", bufs=3) as gpool,
            tc.tile_pool(name="work", bufs=3) as wpool,
            tc.tile_pool(name="psum", bufs=3, space="PSUM") as ppool,
        ):
            w1t_sb = cpool.tile([D, D], f32, tag="w1t")
            nc.sync.dma_start(out=w1t_sb[:], in_=w1t_t[:])
            w2t_sb = cpool.tile([D, D], f32, tag="w2t")
            nc.sync.dma_start(out=w2t_sb[:], in_=w2t_t[:])
            b1_sb = cpool.tile([P, 1], f32, tag="b1")
            nc.sync.dma_start(out=b1_sb[:], in_=b1_t[:])
            b2_sb = cpool.tile([P, 1], f32, tag="b2")
            nc.sync.dma_start(out=b2_sb[:], in_=b2_t[:])

            def relu_bias(out_ap, in_ap, bias_sb, on_act: bool):
                if on_act:
                    nc.scalar.activation(out_ap, in_ap,
                                         mybir.ActivationFunctionType.Relu,
                                         bias=bias_sb[:, 0:1])
                else:
                    nc.vector.tensor_scalar(
                        out=out_ap, in0=in_ap, scalar1=bias_sb[:, 0:1],
                        scalar2=0.0, op0=mybir.AluOpType.add,
                        op1=mybir.AluOpType.max)

            def mmcast(ap):
                return ap.bitcast(mybir.dt.float32r) if use_f32r else ap

            # small first chunk so mm1 starts as early as possible
            chunks = [g] + [chunk_blocks] * ((blocks - g) // chunk_blocks)
            rem = blocks - sum(chunks)
            assert rem % g == 0
            if rem:
                chunks.append(rem)

            st = 0
            r0 = 0
            for cb in chunks:
                win = gpool.tile([P, chunk_blocks * D], f32, tag="win")
                nc.sync.dma_start(
                    out=win[:, : cb * D], in_=tsl_t[:, r0 : r0 + cb * P])
                for s in range(cb // g):
                    r0s = r0 + s * fw
                    ceT = win[:, s * fw : (s + 1) * fw]

                    h_ps = ppool.tile([P, fw], f32, tag="h")
                    nc.tensor.matmul(out=h_ps[:], lhsT=mmcast(w1t_sb[:]),
                                     rhs=mmcast(ceT), start=True, stop=True)
                    hT_sb = wpool.tile([P, fw], f32, tag="hT")
                    relu_bias(hT_sb[:], h_ps[:], b1_sb, on_act=(st % 2 == 0))

                    o_ps = ppool.tile([P, fw], f32, tag="o")
                    nc.tensor.matmul(out=o_ps[:], lhsT=mmcast(w2t_sb[:]),
                                     rhs=mmcast(hT_sb[:]), start=True,
                                     stop=True)
                    o_sb = wpool.tile([P, fw], f32, tag="o_sb")
                    relu_bias(o_sb[:], o_ps[:], b2_sb, on_act=(st % 2 == 1))
                    st += 1

                    nc.sync.dma_start(out=out_t[:, r0s : r0s + fw],
                                      in_=o_sb[:])
                r0 += cb * P

    nc.compile()
    return nc


_CACHED_NC = None
LAST_RESULTS = None


def _get_nc():
    global _CACHED_NC
    if _CACHED_NC is None:
        _CACHED_NC = build_nc(
            use_f32r=os.environ.get("BASS_KERNEL_F32R", "0") == "1")
    return _CACHED_NC


def kernel(nodes, c2e_weight, w1, b1, w2, b2):
    nodes = np.asarray(nodes).astype(np.int64)
    c2e_weight = np.asarray(c2e_weight, dtype=np.float32)
    w1 = np.asarray(w1, dtype=np.float32)
    b1 = np.asarray(b1, dtype=np.float32)
    w2 = np.asarray(w2, dtype=np.float32)
    b2 = np.asarray(b2, dtype=np.float32)

    vocab = c2e_weight.shape[0]
    assert vocab == VOCAB, vocab
    rows = BLOCKS * P  # 12800

    tableT = np.ascontiguousarray(c2e_weight.T)  # [128, VOCAB], d-major

    w1t = np.ascontiguousarray(w1.T)
    w2t = np.ascontiguousarray(w2.T)
    b1c = np.ascontiguousarray(b1.reshape(P, 1))
    b2c = np.ascontiguousarray(b2.reshape(P, 1))

    starts = []
    in_maps = []
    for i in range(N_CORES):
        start = min(i * RANGE, vocab - rows)
        starts.append(start)
        in_maps.append({
            "tslice": np.ascontiguousarray(tableT[:, start : start + rows]),
            "w1t": w1t,
            "w2t": w2t,
            "b1c": b1c,
            "b2c": b2c,
        })

    nc = _get_nc()
    trace = os.environ.get("BASS_KERNEL_TRACE") == "1"
    if trace:
        try:  # tracing needs the NTFF hook; degrade silently without it
            import antenv.axon_hooks  # noqa: F401
        except ImportError:
            trace = False
    res = run_bass_kernel_spmd(nc, in_maps, core_ids=list(range(N_CORES)),
                               trace=trace)
    global LAST_RESULTS
    LAST_RESULTS = res

    # T2[v] = MLP(c2e_weight[v]) assembled from the 8 windows
    t2 = np.empty((vocab, D), dtype=np.float32)
    for i in range(N_CORES):
        dense = res.results[i]["out"]                    # [128, rows] (k, r)
        lo = i * RANGE
        hi = min((i + 1) * RANGE, vocab)
        t2[lo:hi] = dense[:, lo - starts[i] : hi - starts[i]].T

    return t2[nodes]



# revision 4
# speedup vs baseline: 1.7877x; 1.7877x over previous
"""Trainium2 Bass kernel for the Context Encoder problem:

    ce  = c2e_weight[nodes]            # [N, 128] embedding gather
    h   = relu(ce @ w1.T + b1)         # [N, 128]
    out = relu(h @ w2.T + b2)          # [N, 128]

Strategy (8 NeuronCores, vocab-range sharding):
  200000 node ids over a 100000-row vocab saturate every vocab window,
  so transforming the table itself is less work than gathering per-node
  rows (and avoids the per-index DMA descriptor-generation cost that
  dominates any on-device gather).

  - The vocab is split into 8 fixed 12500-row ranges.  Core i streams
    its host-pre-transposed (d-major) table window [128, 12800]
    contiguously at full DMA bandwidth and computes
    T2 = relu(relu(win @ w1.T + b1) @ w2.T + b2) for every window row.
  - Everything on the wire and through the PE is bf16: halves the HBM
    traffic (the memory-regime bound) and runs the PE at 1 cycle/row
    instead of fp32's 4.  PSUM accumulation stays fp32; biases stay
    fp32.  Norm rel-err lands ~5e-3, well inside the 2e-2 gate.
  - relu+bias fuses into one instruction per 1024-wide group
    (2 PSUM banks wide, amortizing the ~360ns per-instruction engine
    latency) and alternates between the ACT and DVE engines (Pool has
    no PSUM access) so neither gates the DMA-bound pipeline.
  - Results stay feature-major; the host transposes each window and maps
    node positions to rows (out = T2[nodes]) as the unshard step.
"""

import sys

for _p in ("/opt/trn_rl_repo",):
    if _p not in sys.path:
        sys.path.insert(0, _p)

import ml_dtypes
import numpy as np

import concourse.bass as bass
import concourse.mybir as mybir
from concourse import bacc
from concourse.bass_utils import run_bass_kernel_spmd
from concourse.tile import TileContext

P = 128
D = 128
N_CORES = 8
VOCAB = 100000
RANGE = VOCAB // N_CORES   # 12500 vocab rows owned per core
BLOCKS = 100               # 12800 rows processed per core (128*100)
FIRST = 4                  # blocks in the pipeline-priming first chunk
CHUNK = 16                 # blocks per steady-state chunk
GROUP = 8                  # blocks per relu group (1024 cols, 2 PSUM banks)
MMW = 512                  # matmul free width (1 PSUM bank)

BF16 = ml_dtypes.bfloat16


def build_nc(relu_engines: int = 2):
    f32 = mybir.dt.float32
    bf16 = mybir.dt.bfloat16
    nc = bacc.Bacc("TRN2", target_bir_lowering=False, debug=False,
                   num_devices=N_CORES)

    rows = BLOCKS * P
    tsl_t = nc.dram_tensor("tslice", [P, rows], bf16,
                           kind="ExternalInput").ap()
    w1t_t = nc.dram_tensor("w1t", [D, D], bf16, kind="ExternalInput").ap()
    w2t_t = nc.dram_tensor("w2t", [D, D], bf16, kind="ExternalInput").ap()
    b1_t = nc.dram_tensor("b1c", [P, 1], f32, kind="ExternalInput").ap()
    b2_t = nc.dram_tensor("b2c", [P, 1], f32, kind="ExternalInput").ap()
    out_t = nc.dram_tensor("out", [P, rows], bf16,
                           kind="ExternalOutput").ap()

    gw_max = GROUP * D  # 1024

    with TileContext(nc) as tc:
        with (
            tc.tile_pool(name="const", bufs=1) as cpool,
            tc.tile_pool(name="win", bufs=3) as gpool,
            tc.tile_pool(name="hT", bufs=3) as hpool,
            tc.tile_pool(name="outs", bufs=3) as opool,
            tc.tile_pool(name="psum", bufs=2, space="PSUM") as ppool,
        ):
            w1t_sb = cpool.tile([D, D], bf16, tag="w1t")
            nc.sync.dma_start(out=w1t_sb[:], in_=w1t_t[:])
            w2t_sb = cpool.tile([D, D], bf16, tag="w2t")
            nc.sync.dma_start(out=w2t_sb[:], in_=w2t_t[:])
            b1_sb = cpool.tile([P, 1], f32, tag="b1")
            nc.sync.dma_start(out=b1_sb[:], in_=b1_t[:])
            b2_sb = cpool.tile([P, 1], f32, tag="b2")
            nc.sync.dma_start(out=b2_sb[:], in_=b2_t[:])

            st = [0]

            def relu_bias(out_ap, in_ap, bias_sb):
                e = st[0] % relu_engines
                st[0] += 1
                if e == 0:
                    nc.scalar.activation(out_ap, in_ap,
                                         mybir.ActivationFunctionType.Relu,
                                         bias=bias_sb[:, 0:1])
                else:
                    eng = nc.vector if e == 1 else nc.gpsimd
                    eng.tensor_scalar(
                        out=out_ap, in0=in_ap, scalar1=bias_sb[:, 0:1],
                        scalar2=0.0, op0=mybir.AluOpType.add,
                        op1=mybir.AluOpType.max)

            chunks = [FIRST] + [CHUNK] * ((BLOCKS - FIRST) // CHUNK)
            assert sum(chunks) == BLOCKS

            r0 = 0  # column offset into the [P, rows] window
            for cb in chunks:
                cw = cb * D
                win = gpool.tile([P, CHUNK * D], bf16, tag="win")
                nc.sync.dma_start(out=win[:, :cw],
                                  in_=tsl_t[:, r0 : r0 + cw])
                out_sb = opool.tile([P, CHUNK * D], bf16, tag="out")

                for g0 in range(0, cb, GROUP):
                    gb = min(GROUP, cb - g0)
                    gw = gb * D
                    goff = g0 * D

                    h_ps = ppool.tile([P, gw_max], f32, tag="h")
                    for s in range(0, gw, MMW):
                        nc.tensor.matmul(
                            out=h_ps[:, s : s + MMW],
                            lhsT=w1t_sb[:],
                            rhs=win[:, goff + s : goff + s + MMW],
                            start=True, stop=True)
                    hT_sb = hpool.tile([P, gw_max], bf16, tag="hT")
                    relu_bias(hT_sb[:, :gw], h_ps[:, :gw], b1_sb)

                    o_ps = ppool.tile([P, gw_max], f32, tag="o")
                    for s in range(0, gw, MMW):
                        nc.tensor.matmul(
                            out=o_ps[:, s : s + MMW],
                            lhsT=w2t_sb[:],
                            rhs=hT_sb[:, s : s + MMW],
                            start=True, stop=True)
                    relu_bias(out_sb[:, goff : goff + gw], o_ps[:, :gw],
                              b2_sb)

                nc.sync.dma_start(out=out_t[:, r0 : r0 + cw],
                                  in_=out_sb[:, :cw])
                r0 += cw

    nc.compile()
    return nc


_CACHED_NC = None
LAST_RESULTS = None


def _get_nc():
    global _CACHED_NC
    if _CACHED_NC is None:
        _CACHED_NC = build_nc()
    return _CACHED_NC


def kernel(nodes, c2e_weight, w1, b1, w2, b2):
    import os

    nodes = np.asarray(nodes).astype(np.int64)
    c2e_weight = np.asarray(c2e_weight, dtype=np.float32)
    w1 = np.asarray(w1, dtype=np.float32)
    b1 = np.asarray(b1, dtype=np.float32)
    w2 = np.asarray(w2, dtype=np.float32)
    b2 = np.asarray(b2, dtype=np.float32)

    vocab = c2e_weight.shape[0]
    assert vocab == VOCAB, vocab
    rows = BLOCKS * P  # 12800

    tableT = np.ascontiguousarray(c2e_weight.T).astype(BF16)  # [128, V]

    w1t = np.ascontiguousarray(w1.T).astype(BF16)
    w2t = np.ascontiguousarray(w2.T).astype(BF16)
    b1c = np.ascontiguousarray(b1.reshape(P, 1))
    b2c = np.ascontiguousarray(b2.reshape(P, 1))

    starts = []
    in_maps = []
    for i in range(N_CORES):
        start = min(i * RANGE, vocab - rows)
        starts.append(start)
        in_maps.append({
            "tslice": np.ascontiguousarray(tableT[:, start : start + rows]),
            "w1t": w1t,
            "w2t": w2t,
            "b1c": b1c,
            "b2c": b2c,
        })

    nc = _get_nc()
    trace = os.environ.get("BASS_KERNEL_TRACE") == "1"
    if trace:
        try:  # tracing needs the NTFF hook; degrade silently without it
            import antenv.axon_hooks  # noqa: F401
        except ImportError:
            trace = False
    res = run_bass_kernel_spmd(nc, in_maps, core_ids=list(range(N_CORES)),
                               trace=trace)
    global LAST_RESULTS
    LAST_RESULTS = res

    # T2[v] = MLP(c2e_weight[v]) assembled from the 8 windows
    t2 = np.empty((vocab, D), dtype=np.float32)
    for i in range(N_CORES):
        dense = res.results[i]["out"]                    # [128, rows] (k, r)
        lo = i * RANGE
        hi = min((i + 1) * RANGE, vocab)
        t2[lo:hi] = dense[:, lo - starts[i] : hi - starts[i]].T

    return t2[nodes]


# revision 5
# speedup vs baseline: 1.8221x; 1.0192x over previous
"""Trainium2 Bass kernel for the Context Encoder problem:

    ce  = c2e_weight[nodes]            # [N, 128] embedding gather
    h   = relu(ce @ w1.T + b1)         # [N, 128]
    out = relu(h @ w2.T + b2)          # [N, 128]

Strategy (8 NeuronCores, vocab-range sharding):
  200000 node ids over a 100000-row vocab saturate every vocab window,
  so transforming the table itself is less work than gathering per-node
  rows.  Core i streams its host-pre-transposed (d-major) window
  [128, 12544] and computes T2 = relu(relu(win@w1.T+b1)@w2.T+b2) for
  every window row; the host maps node positions to rows (out =
  T2[nodes]) as the unshard step.

  Perf structure (memory regime, ~360GB/s/core HBM):
  - bf16 on the wire and through the PE: halves HBM traffic, and the
    PE runs 1 cycle/row instead of fp32's 4.  PSUM stays fp32.
  - Groups of 8 blocks (1024 cols, 2 PSUM banks) flow through a
    software pipeline with mm1 issued one group ahead, so the PE never
    stalls on the relu_h -> mm2 dependency.
  - relu+bias is one fused instruction per group per layer, alternated
    ACT/DVE (Pool has no PSUM access) so both engines stay under the
    PE pace.
  - Weights + output DMAs ride the Scalar-engine HWDGE queue; the
    table input stream rides the SP queue, fully buffered in SBUF so
    the DMA engines are never descriptor-starved.  A 1-block first
    chunk gets the PE started as early as possible; a 1-block last
    chunk keeps the output tail short.
"""

import sys

for _p in ("/opt/trn_rl_repo",):
    if _p not in sys.path:
        sys.path.insert(0, _p)

import ml_dtypes
import numpy as np

import concourse.bass as bass
import concourse.mybir as mybir
from concourse import bacc
from concourse.bass_utils import run_bass_kernel_spmd
from concourse.tile import TileContext

P = 128
D = 128
N_CORES = 8
VOCAB = 100000
RANGE = VOCAB // N_CORES   # 12500 vocab rows owned per core
BLOCKS = 98                # 12544 rows processed per core (128*98)
CHUNK = 16                 # blocks per steady-state input chunk
GROUP = 8                  # blocks per relu group (1024 cols, 2 PSUM banks)
MMW = 512                  # matmul free width (1 PSUM bank)

BF16 = ml_dtypes.bfloat16


def build_nc():
    f32 = mybir.dt.float32
    bf16 = mybir.dt.bfloat16
    nc = bacc.Bacc("TRN2", target_bir_lowering=False, debug=False,
                   num_devices=N_CORES)

    rows = BLOCKS * P
    tsl_t = nc.dram_tensor("tslice", [P, rows], bf16,
                           kind="ExternalInput").ap()
    w1t_t = nc.dram_tensor("w1t", [D, D], bf16, kind="ExternalInput").ap()
    w2t_t = nc.dram_tensor("w2t", [D, D], bf16, kind="ExternalInput").ap()
    b1_t = nc.dram_tensor("b1c", [P, 1], f32, kind="ExternalInput").ap()
    b2_t = nc.dram_tensor("b2c", [P, 1], f32, kind="ExternalInput").ap()
    out_t = nc.dram_tensor("out", [P, rows], bf16,
                           kind="ExternalOutput").ap()

    chunks = [1] + [CHUNK] * ((BLOCKS - 2) // CHUNK) + [1]
    assert sum(chunks) == BLOCKS
    gw_max = GROUP * D  # 1024

    # groups: (chunk_idx, offset within chunk tile, width, global col off)
    groups = []
    r0 = 0
    for ci, cb in enumerate(chunks):
        for g0 in range(0, cb, GROUP):
            gb = min(GROUP, cb - g0)
            groups.append((ci, g0 * D, gb * D, r0 + g0 * D))
        r0 += cb * D

    with TileContext(nc) as tc:
        with (
            tc.tile_pool(name="const", bufs=1) as cpool,
            tc.tile_pool(name="win", bufs=len(chunks)) as gpool,
            tc.tile_pool(name="hT", bufs=3) as hpool,
            tc.tile_pool(name="outs", bufs=4) as opool,
            tc.tile_pool(name="psum", bufs=2, space="PSUM") as ppool,
        ):
            # weights + biases ride the Scalar HWDGE queue so they land
            # ahead of (and parallel to) the SP-queue table stream
            w1t_sb = cpool.tile([D, D], bf16, tag="w1t")
            nc.scalar.dma_start(out=w1t_sb[:], in_=w1t_t[:])
            w2t_sb = cpool.tile([D, D], bf16, tag="w2t")
            nc.scalar.dma_start(out=w2t_sb[:], in_=w2t_t[:])
            b1_sb = cpool.tile([P, 1], f32, tag="b1")
            nc.scalar.dma_start(out=b1_sb[:], in_=b1_t[:])
            b2_sb = cpool.tile([P, 1], f32, tag="b2")
            nc.scalar.dma_start(out=b2_sb[:], in_=b2_t[:])

            # the whole input window fits in SBUF: queue every chunk's
            # DMA upfront so the engines stream back-to-back
            win_tiles = []
            r0 = 0
            for cb in chunks:
                cw = cb * D
                win = gpool.tile([P, CHUNK * D], bf16, tag="win")
                nc.sync.dma_start(out=win[:, :cw],
                                  in_=tsl_t[:, r0 : r0 + cw])
                win_tiles.append(win)
                r0 += cw

            def relu_bias(out_ap, in_ap, bias_sb, on_act):
                if on_act:
                    nc.scalar.activation(out_ap, in_ap,
                                         mybir.ActivationFunctionType.Relu,
                                         bias=bias_sb[:, 0:1])
                else:
                    nc.vector.tensor_scalar(
                        out=out_ap, in0=in_ap, scalar1=bias_sb[:, 0:1],
                        scalar2=0.0, op0=mybir.AluOpType.add,
                        op1=mybir.AluOpType.max)

            h_tiles = {}

            def mm1(gi):
                ci, goff, gw, _ = groups[gi]
                win = win_tiles[ci]
                h_ps = ppool.tile([P, gw_max], f32, tag="h")
                for s in range(0, gw, MMW):
                    w = min(MMW, gw - s)
                    nc.tensor.matmul(
                        out=h_ps[:, s : s + w], lhsT=w1t_sb[:],
                        rhs=win[:, goff + s : goff + s + w],
                        start=True, stop=True)
                h_tiles[gi] = h_ps

            def rest(gi):
                _, _, gw, r0c = groups[gi]
                h_ps = h_tiles.pop(gi)
                hT_sb = hpool.tile([P, gw_max], bf16, tag="hT")
                relu_bias(hT_sb[:, :gw], h_ps[:, :gw], b1_sb,
                          on_act=(gi % 2 == 0))
                o_ps = ppool.tile([P, gw_max], f32, tag="o")
                for s in range(0, gw, MMW):
                    w = min(MMW, gw - s)
                    nc.tensor.matmul(
                        out=o_ps[:, s : s + w], lhsT=w2t_sb[:],
                        rhs=hT_sb[:, s : s + w], start=True, stop=True)
                out_sb = opool.tile([P, gw_max], bf16, tag="out")
                relu_bias(out_sb[:, :gw], o_ps[:, :gw], b2_sb,
                          on_act=(gi % 2 == 1))
                nc.scalar.dma_start(out=out_t[:, r0c : r0c + gw],
                                    in_=out_sb[:, :gw])

            mm1(0)
            for gi in range(len(groups)):
                if gi + 1 < len(groups):
                    mm1(gi + 1)
                rest(gi)

    nc.compile()
    return nc


_CACHED_NC = None
LAST_RESULTS = None


def _get_nc():
    global _CACHED_NC
    if _CACHED_NC is None:
        _CACHED_NC = build_nc()
    return _CACHED_NC


def kernel(nodes, c2e_weight, w1, b1, w2, b2):
    import os

    nodes = np.asarray(nodes).astype(np.int64)
    c2e_weight = np.asarray(c2e_weight, dtype=np.float32)
    w1 = np.asarray(w1, dtype=np.float32)
    b1 = np.asarray(b1, dtype=np.float32)
    w2 = np.asarray(w2, dtype=np.float32)
    b2 = np.asarray(b2, dtype=np.float32)

    vocab = c2e_weight.shape[0]
    assert vocab == VOCAB, vocab
    rows = BLOCKS * P  # 12544

    tableT = np.ascontiguousarray(c2e_weight.T).astype(BF16)  # [128, V]

    w1t = np.ascontiguousarray(w1.T).astype(BF16)
    w2t = np.ascontiguousarray(w2.T).astype(BF16)
    b1c = np.ascontiguousarray(b1.reshape(P, 1))
    b2c = np.ascontiguousarray(b2.reshape(P, 1))

    starts = []
    in_maps = []
    for i in range(N_CORES):
        start = min(i * RANGE, vocab - rows)
        starts.append(start)
        in_maps.append({
            "tslice": np.ascontiguousarray(tableT[:, start : start + rows]),
            "w1t": w1t,
            "w2t": w2t,
            "b1c": b1c,
            "b2c": b2c,
        })

    nc = _get_nc()
    trace = os.environ.get("BASS_KERNEL_TRACE") == "1"
    if trace:
        try:  # tracing needs the NTFF hook; degrade silently without it
            import antenv.axon_hooks  # noqa: F401
        except ImportError:
            trace = False
    res = run_bass_kernel_spmd(nc, in_maps, core_ids=list(range(N_CORES)),
                               trace=trace)
    global LAST_RESULTS
    LAST_RESULTS = res

    # T2[v] = MLP(c2e_weight[v]) assembled from the 8 windows
    t2 = np.empty((vocab, D), dtype=np.float32)
    for i in range(N_CORES):
        dense = res.results[i]["out"]                    # [128, rows] (k, r)
        lo = i * RANGE
        hi = min((i + 1) * RANGE, vocab)
        t2[lo:hi] = dense[:, lo - starts[i] : hi - starts[i]].T

    return t2[nodes]


# revision 12
# speedup vs baseline: 1.8658x; 1.0240x over previous
"""Trainium2 Bass kernel for the Context Encoder problem:

    ce  = c2e_weight[nodes]            # [N, 128] embedding gather
    h   = relu(ce @ w1.T + b1)         # [N, 128]
    out = relu(h @ w2.T + b2)          # [N, 128]

Strategy (8 NeuronCores, vocab-range sharding):
  200000 node ids over a 100000-row vocab saturate every vocab window,
  so transforming the table itself is less work than gathering per-node
  rows.  Core i streams its host-pre-transposed (d-major) window
  [128, 12544] and computes T2 = relu(relu(win@w1.T+b1)@w2.T+b2) for
  every window row; the host maps node positions to rows (out =
  T2[nodes]) as the unshard step.

  Perf structure (memory regime, ~360GB/s/core HBM):
  - bf16 on the wire and through the PE: halves HBM traffic, and the
    PE runs 1 cycle/row instead of fp32's 4.  PSUM stays fp32.
  - Groups of 8 blocks (1024 cols, 2 PSUM banks) flow through a
    software pipeline with mm1 issued one group ahead, so the PE never
    stalls on the relu_h -> mm2 dependency.
  - relu+bias is one fused instruction per group per layer, alternated
    ACT/DVE (Pool has no PSUM access) so both engines stay under the
    PE pace.
  - Weights + output DMAs ride the Scalar-engine HWDGE queue; the
    table input stream rides the SP queue, fully buffered in SBUF so
    the DMA engines are never descriptor-starved.  A 1-block first
    chunk gets the PE started as early as possible; a 1-block last
    chunk keeps the output tail short.
"""

import sys

for _p in ("/opt/trn_rl_repo",):
    if _p not in sys.path:
        sys.path.insert(0, _p)

import ml_dtypes
import numpy as np

import concourse.bass as bass
import concourse.mybir as mybir
from concourse import bacc
from concourse.bass_utils import run_bass_kernel_spmd
from concourse.tile import TileContext

P = 128
D = 128
N_CORES = 8
VOCAB = 100000
RANGE = VOCAB // N_CORES   # 12500 vocab rows owned per core
BLOCKS = 98                # 12544 rows processed per core (128*98)
GROUP = 8                  # blocks per relu group (1024 cols, 2 PSUM banks)
MMW = 512                  # matmul free width (1 PSUM bank)

# input DMA chunks (blocks): tiny first chunks prime the compute
# pipeline, then steady 16-block transfers (DMA triggers cost ~600ns
# of issuing-engine time each, and a group's matmul waits on its whole
# chunk — so chunks grow as the pipeline fills).
CHUNKS = [1, 4, 8, 16, 16, 16, 16, 21]
assert sum(CHUNKS) == BLOCKS

BF16 = ml_dtypes.bfloat16


def build_nc():
    f32 = mybir.dt.float32
    bf16 = mybir.dt.bfloat16
    nc = bacc.Bacc("TRN2", target_bir_lowering=False, debug=False,
                   num_devices=N_CORES)

    rows = BLOCKS * P
    tsl_t = nc.dram_tensor("tslice", [P, rows], bf16,
                           kind="ExternalInput").ap()
    wb_t = nc.dram_tensor("wb", [D, 2 * D], bf16, kind="ExternalInput").ap()
    b12_t = nc.dram_tensor("b12", [P, 2], f32, kind="ExternalInput").ap()
    out_t = nc.dram_tensor("out", [P, rows], bf16,
                           kind="ExternalOutput").ap()

    gw_max = GROUP * D  # 1024

    # groups: (chunk_idx, offset within chunk tile, width, global col off)
    groups = []
    r0 = 0
    for ci, cb in enumerate(CHUNKS):
        for g0 in range(0, cb, GROUP):
            gb = min(GROUP, cb - g0)
            groups.append((ci, g0 * D, gb * D, r0 + g0 * D))
        r0 += cb * D
    n_g = len(groups)

    # output DMA batches: consecutive groups, <= 16 blocks per batch,
    # last batch kept to one group so the tail DMA is short
    batches = []
    cur = []
    cur_w = 0
    for gi, (_, _, gw, _) in enumerate(groups):
        if cur and (cur_w + gw > 16 * D or gi == n_g - 1):
            batches.append(cur)
            cur = []
            cur_w = 0
        cur.append(gi)
        cur_w += gw
    batches.append(cur)
    batch_of = {}          # group -> (batch_idx, last-in-batch?)
    for bi, b in enumerate(batches):
        for gi in b:
            batch_of[gi] = (bi, gi == b[-1])

    with TileContext(nc) as tc:
        with (
            tc.tile_pool(name="const", bufs=1) as cpool,
            tc.tile_pool(name="winp", bufs=1) as gpool,
            tc.tile_pool(name="hT", bufs=3) as hpool,
            tc.tile_pool(name="outs", bufs=1) as opool,
            tc.tile_pool(name="psum", bufs=2, space="PSUM") as ppool,
        ):
            # weights ride in front of the table stream on the SP queue;
            # biases take the Scalar queue in parallel
            wb_sb = cpool.tile([D, 2 * D], bf16, tag="wb")
            nc.sync.dma_start(out=wb_sb[:], in_=wb_t[:])
            b12_sb = cpool.tile([P, 2], f32, tag="b12")
            nc.scalar.dma_start(out=b12_sb[:], in_=b12_t[:])
            w1t_sb = wb_sb[:, 0:D]
            w2t_sb = wb_sb[:, D : 2 * D]
            b1_sb = b12_sb[:, 0:1]
            b2_sb = b12_sb[:, 1:2]

            # the whole input window fits in SBUF: queue every chunk's
            # DMA upfront so the engines stream back-to-back
            win_tiles = []
            r0 = 0
            for ci, cb in enumerate(CHUNKS):
                cw = cb * D
                win = gpool.tile([P, cw], bf16, tag=f"win{ci}")
                nc.sync.dma_start(out=win[:], in_=tsl_t[:, r0 : r0 + cw])
                win_tiles.append(win)
                r0 += cw

            # one SBUF tile per output batch (sum = whole window, fits)
            out_tiles = []
            for bi, b in enumerate(batches):
                bw = sum(groups[gi][2] for gi in b)
                out_tiles.append(opool.tile([P, bw], bf16,
                                            name=f"outsb{bi}",
                                            tag=f"out{bi}"))

            def relu_bias(out_ap, in_ap, bias_sb, on_act):
                if on_act:
                    nc.scalar.activation(out_ap, in_ap,
                                         mybir.ActivationFunctionType.Relu,
                                         bias=bias_sb)
                else:
                    nc.vector.tensor_scalar(
                        out=out_ap, in0=in_ap, scalar1=bias_sb,
                        scalar2=0.0, op0=mybir.AluOpType.add,
                        op1=mybir.AluOpType.max)

            h_tiles = {}

            def mm1(gi):
                ci, goff, gw, _ = groups[gi]
                win = win_tiles[ci]
                h_ps = ppool.tile([P, gw_max], f32, tag="h")
                for s in range(0, gw, MMW):
                    w = min(MMW, gw - s)
                    nc.tensor.matmul(
                        out=h_ps[:, s : s + w], lhsT=w1t_sb,
                        rhs=win[:, goff + s : goff + s + w],
                        start=True, stop=True)
                h_tiles[gi] = h_ps

            def rest(gi):
                _, _, gw, r0c = groups[gi]
                h_ps = h_tiles.pop(gi)
                hT_sb = hpool.tile([P, gw_max], bf16, tag="hT")
                relu_bias(hT_sb[:, :gw], h_ps[:, :gw], b1_sb,
                          on_act=(gi % 2 == 0))
                o_ps = ppool.tile([P, gw_max], f32, tag="o")
                for s in range(0, gw, MMW):
                    w = min(MMW, gw - s)
                    nc.tensor.matmul(
                        out=o_ps[:, s : s + w], lhsT=w2t_sb,
                        rhs=hT_sb[:, s : s + w], start=True, stop=True)
                bi, last = batch_of[gi]
                bstart = groups[batches[bi][0]][3]
                out_sb = out_tiles[bi]
                boff = r0c - bstart
                relu_bias(out_sb[:, boff : boff + gw], o_ps[:, :gw], b2_sb,
                          on_act=(gi % 2 == 1))
                if last:
                    bw = sum(groups[g][2] for g in batches[bi])
                    # early batches queue behind the input stream on SP;
                    # the final two ride the Scalar queue so the tail
                    # isn't blocked behind queued input packets
                    eng = nc.scalar if bi >= len(batches) - 2 else nc.sync
                    eng.dma_start(out=out_t[:, bstart : bstart + bw],
                                  in_=out_sb[:])

            mm1(0)
            for gi in range(n_g):
                if gi + 1 < n_g:
                    mm1(gi + 1)
                rest(gi)

    nc.compile()
    return nc


_CACHED_NC = None
LAST_RESULTS = None


def _get_nc():
    global _CACHED_NC
    if _CACHED_NC is None:
        _CACHED_NC = build_nc()
    return _CACHED_NC


def kernel(nodes, c2e_weight, w1, b1, w2, b2):
    import os

    nodes = np.asarray(nodes).astype(np.int64)
    c2e_weight = np.asarray(c2e_weight, dtype=np.float32)
    w1 = np.asarray(w1, dtype=np.float32)
    b1 = np.asarray(b1, dtype=np.float32)
    w2 = np.asarray(w2, dtype=np.float32)
    b2 = np.asarray(b2, dtype=np.float32)

    vocab = c2e_weight.shape[0]
    assert vocab == VOCAB, vocab
    rows = BLOCKS * P  # 12544

    tableT = np.ascontiguousarray(c2e_weight.T).astype(BF16)  # [128, V]

    wb = np.ascontiguousarray(
        np.concatenate([w1.T, w2.T], axis=1)).astype(BF16)    # [128, 256]
    b12 = np.ascontiguousarray(
        np.stack([b1, b2], axis=1).astype(np.float32))        # [128, 2]

    starts = []
    in_maps = []
    for i in range(N_CORES):
        start = min(i * RANGE, vocab - rows)
        starts.append(start)
        in_maps.append({
            "tslice": np.ascontiguousarray(tableT[:, start : start + rows]),
            "wb": wb,
            "b12": b12,
        })

    nc = _get_nc()
    trace = os.environ.get("BASS_KERNEL_TRACE") == "1"
    if trace:
        try:  # tracing needs the NTFF hook; degrade silently without it
            import antenv.axon_hooks  # noqa: F401
        except ImportError:
            trace = False
    res = run_bass_kernel_spmd(nc, in_maps, core_ids=list(range(N_CORES)),
                               trace=trace)
    global LAST_RESULTS
    LAST_RESULTS = res

    # T2[v] = MLP(c2e_weight[v]) assembled from the 8 windows
    t2 = np.empty((vocab, D), dtype=np.float32)
    for i in range(N_CORES):
        dense = res.results[i]["out"]                    # [128, rows] (k, r)
        lo = i * RANGE
        hi = min((i + 1) * RANGE, vocab)
        t2[lo:hi] = dense[:, lo - starts[i] : hi - starts[i]].T

    return t2[nodes]
